# revision 1
# baseline (speedup 1.0000x reference)
"""Differentiable-JPEG Trainium2 kernel (8-core data-parallel, full I/O).

Pipeline per 32-row x 512-col x 3-channel tile (96 packed partitions):
  MM1 (x4):  T = X^T @ A1aug      color transform + H-DCT (+DC bias via
                                  augmented ones row), per 128-col chunk
  MM2:       C = BDt^T @ T        W-DCT (block-diag D^T), one matmul
  quant:     t = tanh(RQ15*C)  (round(d)==0 exactly since |d|<0.5);
             Cq = C + t*HQ
  MM3 (x4):  R = Cq^T @ BD        W-IDCT
  MM4 (x4):  Y = AIaug^T @ R''    H-IDCT + inverse color + 0.5 bias
  clip:      out = min(max(Y,0),1)

Batch dim (32) sharded 4-per-core across 8 NeuronCores; constants
replicated. All matmul biases ride augmented contraction rows (ones rows
kept in persistent SBUF buffers).
"""
import numpy as np

B, C, H, W = 32, 3, 512, 512
NCORES = 8
BPC = B // NCORES           # images per core
G, CCH, XX = 4, 3, 8        # 8-row groups per tile, channels, rows per block
P96 = G * CCH * XX          # 96 packed partitions
NT = H // 32                # 16 h-tiles per image
FREE = NT * W               # 8192 free elements per image buffer
MAGIC = 12582912.0          # 1.5*2^23 fp32 round-to-nearest-even trick

QUALITY = 50.0
_LUM = np.array([[16,11,10,16,24,40,51,61],[12,12,14,19,26,58,60,55],[14,13,16,24,40,57,69,56],[14,17,22,29,51,87,80,62],[18,22,37,56,68,109,103,77],[24,35,55,64,81,104,113,92],[49,64,78,87,103,121,120,101],[72,92,95,98,112,100,103,99]], dtype=np.float32)
_CHR = np.array([[17,18,24,47,99,99,99,99],[18,21,26,66,99,99,99,99],[24,26,56,99,99,99,99,99],[47,66,99,99,99,99,99,99],[99,99,99,99,99,99,99,99],[99,99,99,99,99,99,99,99],[99,99,99,99,99,99,99,99],[99,99,99,99,99,99,99,99]], dtype=np.float32)


def _scaled_qtable(base, qf):
    qf = max(1.0, min(100.0, qf))
    s = 5000.0 / qf if qf < 50 else 200.0 - 2.0 * qf
    return np.maximum(np.floor((base * s + 50.0) / 100.0), 1.0)


def _np_consts():
    qtab = np.stack([_scaled_qtable(_LUM, QUALITY), _scaled_qtable(_CHR, QUALITY),
                     _scaled_qtable(_CHR, QUALITY)]).astype(np.float32)  # [c,u,v]
    u8 = np.arange(8)[:, None]
    x8 = np.arange(8)[None, :]
    cu = np.where(u8 == 0, 1.0 / np.sqrt(2.0), 1.0)
    D = (0.5 * cu * np.cos((2 * x8 + 1) * u8 * np.pi / 16.0)).astype(np.float32)
    MFWD = np.array([[0.299, 0.587, 0.114], [-0.168736, -0.331264, 0.5],
                     [0.5, -0.418688, -0.081312]], np.float32)
    MINV = np.array([[1.0, 0.0, 1.402], [1.0, -0.344136, -0.714136],
                     [1.0, 1.772, 0.0]], np.float32)

    A1 = np.zeros((97, 96), np.float32)
    AI = np.zeros((97, 96), np.float32)
    for g in range(G):
        for c in range(CCH):
            for c2 in range(CCH):
                p0 = c*32 + g*8
                n0 = c2*32 + g*8
                # A1[p=(c,g,xx), n=(c2,g,u)] = MFWD[c2,c] * D[u,xx]
                A1[p0:p0+8, n0:n0+8] = MFWD[c2, c] * D.T
                # AI[k=(c2,g,u), m=(c,g,xx)] = MINV[c,c2] * D[u,xx]
                AI[n0:n0+8, p0:p0+8] = MINV[c, c2] * D
        A1[96, g*8] = -np.sqrt(2.0)     # (c2=Y, u=0): forward -0.5 pixel bias
    AI[96, :] = 0.5                      # +0.5 pixel bias on inverse

    BDt = np.zeros((128, 128), np.float32)
    for a in range(16):
        BDt[8*a:8*a+8, 8*a:8*a+8] = D.T
    BD = np.ascontiguousarray(BDt.T)

    RQ = np.zeros((128, 384), np.float32)
    HQ = np.zeros((128, 384), np.float32)
    v = np.arange(128) % 8
    for j in range(4):
        for c in range(CCH):
            for g in range(G):
                for u in range(XX):
                    col = j*96 + c*32 + g*8 + u
                    RQ[:, col] = 15.0 / qtab[c, u, v]
                    HQ[:, col] = 0.5 * qtab[c, u, v]
    return {"a1": A1, "ai": AI, "bdt": BDt, "bd": BD, "rq": RQ, "hq": HQ}


_CACHE = {}


def _build(work_bufs=3, rsb_n=3, uu_pool=True, dma_split=2, ablate=(), use_f32r="both", scalar_stores=True, mm3_fp16=False):
    import concourse.bacc as bacc
    import concourse.mybir as mybir
    import concourse.tile as tile

    F32 = mybir.dt.float32
    F32R = mybir.dt.float32r
    AOT = mybir.AluOpType
    nc = bacc.Bacc("TRN2", target_bir_lowering=False, debug=False)

    x = nc.dram_tensor("x", [BPC, C, H, W], F32, kind="ExternalInput")
    out = nc.dram_tensor("out", [BPC, C, H, W], F32, kind="ExternalOutput")
    if use_f32r == "both" or use_f32r is True:
        _r = {"bdt", "ai"}
    elif use_f32r == "mm4":
        _r = {"ai"}
    else:
        _r = set()
    F16 = mybir.dt.float16
    _h = {"bd"} if mm3_fp16 else set()
    cd = {k: nc.dram_tensor(
              k, list(vv.shape),
              F32R if k in _r else (F16 if k in _h else F32),
              kind="ExternalInput")
          for k, vv in _np_consts().items()}

    # per-(image, channel) packed APs: partitions (g xx), free (t, w)
    xin_src = x.ap().rearrange("b c (t g xx) w -> b c (g xx) t w", t=NT, g=G, xx=XX)
    out_dst = out.ap().rearrange("b c (t g xx) w -> b c (g xx) t w", t=NT, g=G, xx=XX)

    with tile.TileContext(nc) as tc:
        # persistent SBUF state
        csb = {k: nc.alloc_sbuf_tensor(
                   f"c_{k}", list(v.shape),
                   F32R if k in _r else (F16 if k in _h else F32))
               for k, v in _np_consts().items()}
        xin = [nc.alloc_sbuf_tensor(f"xin{i}", [97, FREE], F32) for i in range(2)]
        rout = [nc.alloc_sbuf_tensor(f"rout{i}", [P96, FREE], F32) for i in range(2)]
        rsb = [nc.alloc_sbuf_tensor(f"rsb{i}", [97, W], F32R if "ai" in _r else F32) for i in range(rsb_n)]
        zbias = nc.alloc_sbuf_tensor("zbias", [128, 1], F32)

        for k, t in csb.items():
            nc.sync.dma_start(out=t.ap(), in_=cd[k].ap())
        nc.vector.memset(zbias.ap(), 0.0)
        for i in range(2):
            nc.vector.memset(xin[i].ap()[96:97, :], 1.0)
        for i in range(rsb_n):
            nc.vector.memset(rsb[i].ap()[96:97, :].bitcast(F32), 1.0)

        a1, ai = csb["a1"].ap(), csb["ai"].ap()
        bdt, bd = csb["bdt"].ap(), csb["bd"].ap()
        rq, hq = csb["rq"].ap(), csb["hq"].ap()
        zb = zbias.ap()

        with (
            tc.tile_pool(name="psT", bufs=3, space="PSUM") as psT,
            tc.tile_pool(name="psC", bufs=2, space="PSUM") as psC,
            tc.tile_pool(name="psR", bufs=2, space="PSUM") as psR,
            tc.tile_pool(name="psY", bufs=1, space="PSUM") as psY,
            tc.tile_pool(name="work", bufs=work_bufs) as work,
        ):
            tchunk = NT // dma_split

            def load_image(b):
                if "dma" in ablate:
                    return
                xv = xin[b % 2].ap()
                for c in range(CCH):
                    for s0 in range(dma_split):
                        nc.sync.dma_start(
                            out=xv[c*32:(c+1)*32,
                                   s0*tchunk*W:(s0+1)*tchunk*W].rearrange(
                                "p (t w) -> p t w", t=tchunk),
                            in_=xin_src[b, c, :, s0*tchunk:(s0+1)*tchunk])

            def store_half(b, s0):
                if "dma" in ablate:
                    return
                ov = rout[b % 2].ap()
                for c in range(CCH):
                    (nc.scalar if scalar_stores else nc.sync).dma_start(
                        out=out_dst[b, c, :, s0*tchunk:(s0+1)*tchunk],
                        in_=ov[c*32:(c+1)*32,
                               s0*tchunk*W:(s0+1)*tchunk*W].rearrange(
                            "p (t w) -> p t w", t=tchunk))

            # 5-stage software pipeline over the 64 (image, h-tile) items:
            #   it j:   MM1(j)x4 -> T_ps ; T-copy -> t_sb
            #   it j+1: MM2(j) -> C_ps ; d15(j) ; tanh(j)
            #   it j+2: uu(j) ; cq(j)      (C_ps lives 2 iters -> psC bufs 3)
            #   it j+3: MM3(j)x4 -> R_ps ; R-copy -> rsb
            #   it j+4: MM4(j) ; clip(j)
            # Emission order keeps every engine queue dependency-clean so no
            # FIFO stream ever stalls on a same-iteration producer.
            items = [(b, t) for b in range(BPC) for t in range(NT)]
            NI = len(items)
            st = {}   # per-item live tiles

            load_image(0)
            if BPC > 1:
                load_image(1)

            for i in range(NI + 4):
                # PE ops first: all deps are >= 1 iteration old.
                if i < NI:
                    b, t = items[i]
                    xv = xin[b % 2].ap()
                    base = t * W
                    T_ps = psT.tile([128, 384], F32)
                    for j in range(4):
                        nc.tensor.matmul(
                            T_ps[:, 96*j:96*j+96],
                            xv[0:97, base+128*j:base+128*j+128],
                            a1, start=True, stop=True)
                    st[i] = {"T_ps": T_ps, "b": b, "t": t}
                if i - 1 >= 0 and i - 1 < NI:
                    e = st[i - 1]
                    C_ps = psC.tile([128, 384], F32)
                    nc.tensor.matmul(C_ps[:, :], bdt, e["t_sb"],
                                     start=True, stop=True)
                    e["C_ps"] = C_ps
                if i - 3 >= 0 and i - 3 < NI:
                    e = st[i - 3]
                    R_ps = psR.tile([P96, W], F32)
                    for j in range(4):
                        nc.tensor.matmul(
                            R_ps[:, 128*j:128*j+128],
                            e["cq"][:, 96*j:96*j+96],
                            bd, start=True, stop=True)
                    e["R_ps"] = R_ps
                if i - 4 >= 0 and i - 4 < NI:
                    e = st[i - 4]
                    Y_ps = psY.tile([P96, W], F32)
                    nc.tensor.matmul(Y_ps[:, :], ai, e["rv"][0:97, :],
                                     start=True, stop=True)
                    e["Y_ps"] = Y_ps

                # ACT ops
                if i < NI:
                    e = st[i]
                    t_sb = work.tile([128, 384], F32R if "bdt" in _r else F32, tag="t_sb")
                    nc.scalar.copy(t_sb, e["T_ps"][:, :])
                    e["t_sb"] = t_sb
                if i - 1 >= 0 and i - 1 < NI and "quant" not in ablate:
                    e = st[i - 1]
                    # |d| = |C|/q <= 4.0/10 < 0.5 always => round(d) == 0,
                    # so tanh(15(d-round(d))) == tanh(RQ15*C) exactly.
                    d15 = work.tile([128, 384], F32, tag="d15")
                    nc.vector.tensor_tensor(d15, e["C_ps"][:, :], rq, AOT.mult)
                    tt = work.tile([128, 384], F32, tag="tt")
                    nc.scalar.activation(tt, d15,
                                         mybir.ActivationFunctionType.Tanh,
                                         bias=zb, scale=1.0)
                    e["tt"] = tt
                if i - 3 >= 0 and i - 3 < NI:
                    e = st[i - 3]
                    rv = rsb[(i - 3) % rsb_n].ap()
                    nc.scalar.copy(rv[0:P96, :], e["R_ps"][:, :])
                    e["rv"] = rv

                # POOL + DVE quant tail
                if i - 2 >= 0 and i - 2 < NI:
                    e = st[i - 2]
                    if "quant" in ablate:
                        e["cq"] = e["t_sb"].bitcast(F32)
                    else:
                        uu = work.tile([128, 384], F32, tag="uu")
                        (nc.gpsimd if uu_pool else nc.vector).tensor_tensor(
                            uu, e["tt"], hq, AOT.mult)
                        cq = work.tile([128, 384], F16 if mm3_fp16 else F32,
                                       tag="cq")
                        nc.vector.tensor_tensor(cq, e["C_ps"][:, :], uu, AOT.add)
                        e["cq"] = cq
                if i - 4 >= 0 and i - 4 < NI:
                    e = st[i - 4]
                    ov = rout[e["b"] % 2].ap()
                    if "clip" not in ablate:
                        nc.vector.tensor_scalar(
                            ov[:, e["t"]*W:(e["t"]+1)*W], e["Y_ps"][:, :], 0.0, 1.0,
                            AOT.max, AOT.min)
                    if e["t"] == NT - 1:
                        for s0 in range(dma_split):
                            store_half(e["b"], s0)
                    del st[i - 4]

                # prefetch: at the first tile of image b, all of image
                # b-1's MM1s are already emitted, so overwriting
                # xin[(b+1)%2] for image b+1 is safe in program order.
                if i < NI:
                    b, t = items[i]
                    if t == 0 and b >= 1 and b + 1 < BPC:
                        load_image(b + 1)
    nc.compile()
    return nc


def _get_nc(**kw):
    key = tuple(sorted(kw.items()))
    if key not in _CACHE:
        _CACHE[key] = _build(**kw)
    return _CACHE[key]


def kernel(x, trace=False, **kw):
    from concourse import bass_utils
    nc = _get_nc(**kw)
    consts = _np_consts()
    if kw.get("mm3_fp16"):
        consts["bd"] = consts["bd"].astype(np.float16)
    x = np.ascontiguousarray(np.asarray(x), dtype=np.float32)
    in_maps = []
    for i in range(NCORES):
        m = {"x": x[i*BPC:(i+1)*BPC]}
        m.update(consts)
        in_maps.append(m)
    try:
        res = bass_utils.run_bass_kernel_spmd(
            nc, in_maps, core_ids=list(range(NCORES)), trace=trace)
    except Exception:
        if not trace:
            raise
        res = bass_utils.run_bass_kernel_spmd(
            nc, in_maps, core_ids=list(range(NCORES)), trace=False)
    _CACHE["last"] = res
    return np.concatenate([r["out"] for r in res.results], axis=0)


def last_exec_time_ns():
    res = _CACHE.get("last")
    return None if res is None else res.exec_time_ns



# revision 11
# speedup vs baseline: 1.1233x; 1.1233x over previous
"""Differentiable-JPEG Trainium2 kernel (8-core data-parallel, full I/O).

Identity-split formulation: since IDCT2(DCT2(x)) == x and M_INV @ M_FWD == I
(to 6e-7), the reference reduces to

    out = clip(x + M_INV . IDCT2(corr), 0, 1),
    corr = 0.5 * q * tanh(15 * C / q),   C = DCT2(M_FWD.x + OFF - 0.5)

(|C/q| <= 4/10 < 0.5 so round(C/q) == 0 exactly). Only the small correction
term flows through the transform chain, so the chain runs in fp16/f32r; x
re-enters through an identity-weight matmul accumulated into MM4's PSUM,
which deletes the dequant add and all bias plumbing.

The input image is pre-converted fp32->fp16 per image by a single
contiguous HBM->HBM gpsimd DMA (only Pool software-DGE can cast in
flight), then loaded fp16; the fp16 x quantization (2.4e-4) is well
inside the 2e-2 gate and MM1 becomes a legal fp16 matmul.

Pipeline per 32-row x 512-col x 3-channel tile (96 packed partitions):
  MM1 (x4):  T = X^T @ A1        color + H-DCT (fp16)
  MM2:       C = BDt^T @ T       W-DCT (f32r, free 384)
  quant:     d15 = C*RQ (DVE) ; tt = tanh(d15) fp16 (ACT)
             uu = tt*HQ fp16 (DVE 4x mode)
  MM3 (x4):  R = uu^T @ BD       W-IDCT (fp16)
  MM4:       Y = AI^T @ R + I^T @ X   H-IDCT+color-inv plus identity
  clip:      out = min(max(Y,0),1)    column-split DVE/Pool

Batch dim (32) sharded 4-per-core across 8 NeuronCores; constants
replicated. PSUM->SBUF copies and clip are column-split across ACT/DVE/Pool
to balance engine busy time; stores ride the otherwise idle sync engine.
"""
import numpy as np

B, C, H, W = 32, 3, 512, 512
NCORES = 8
BPC = B // NCORES           # images per core
G, CCH, XX = 4, 3, 8        # 8-row groups per tile, channels, rows per block
P96 = G * CCH * XX          # 96 packed partitions
NT = H // 32                # 16 h-tiles per image
FREE = NT * W               # 8192 free elements per image buffer

QUALITY = 50.0
_LUM = np.array([[16,11,10,16,24,40,51,61],[12,12,14,19,26,58,60,55],[14,13,16,24,40,57,69,56],[14,17,22,29,51,87,80,62],[18,22,37,56,68,109,103,77],[24,35,55,64,81,104,113,92],[49,64,78,87,103,121,120,101],[72,92,95,98,112,100,103,99]], dtype=np.float32)
_CHR = np.array([[17,18,24,47,99,99,99,99],[18,21,26,66,99,99,99,99],[24,26,56,99,99,99,99,99],[47,66,99,99,99,99,99,99],[99,99,99,99,99,99,99,99],[99,99,99,99,99,99,99,99],[99,99,99,99,99,99,99,99],[99,99,99,99,99,99,99,99]], dtype=np.float32)


def _scaled_qtable(base, qf):
    qf = max(1.0, min(100.0, qf))
    s = 5000.0 / qf if qf < 50 else 200.0 - 2.0 * qf
    return np.maximum(np.floor((base * s + 50.0) / 100.0), 1.0)


def _np_consts():
    qtab = np.stack([_scaled_qtable(_LUM, QUALITY), _scaled_qtable(_CHR, QUALITY),
                     _scaled_qtable(_CHR, QUALITY)]).astype(np.float32)  # [c,u,v]
    u8 = np.arange(8)[:, None]
    x8 = np.arange(8)[None, :]
    cu = np.where(u8 == 0, 1.0 / np.sqrt(2.0), 1.0)
    D = (0.5 * cu * np.cos((2 * x8 + 1) * u8 * np.pi / 16.0)).astype(np.float32)
    MFWD = np.array([[0.299, 0.587, 0.114], [-0.168736, -0.331264, 0.5],
                     [0.5, -0.418688, -0.081312]], np.float32)
    MINV = np.array([[1.0, 0.0, 1.402], [1.0, -0.344136, -0.714136],
                     [1.0, 1.772, 0.0]], np.float32)

    A1 = np.zeros((97, 96), np.float32)
    AI = np.zeros((96, 96), np.float32)
    for g in range(G):
        for c in range(CCH):
            for c2 in range(CCH):
                p0 = c*32 + g*8
                n0 = c2*32 + g*8
                # A1[p=(c,g,xx), n=(c2,g,u)] = MFWD[c2,c] * D[u,xx]
                A1[p0:p0+8, n0:n0+8] = MFWD[c2, c] * D.T
                # AI[k=(c2,g,u), m=(c,g,xx)] = MINV[c,c2] * D[u,xx]
                AI[n0:n0+8, p0:p0+8] = MINV[c, c2] * D
        A1[96, g*8] = -np.sqrt(2.0)     # (c2=Y, u=0): forward -0.5 pixel bias

    BDt = np.zeros((128, 128), np.float32)
    for a in range(16):
        BDt[8*a:8*a+8, 8*a:8*a+8] = D.T
    BD = np.ascontiguousarray(BDt.T)

    RQ = np.zeros((128, 384), np.float32)
    HQ = np.zeros((128, 384), np.float32)
    v = np.arange(128) % 8
    for j in range(4):
        for c in range(CCH):
            for g in range(G):
                for u in range(XX):
                    col = j*96 + c*32 + g*8 + u
                    RQ[:, col] = 15.0 / qtab[c, u, v]
                    HQ[:, col] = 0.5 * qtab[c, u, v]
    I96 = np.eye(96, dtype=np.float32)
    return {"a1": A1, "ai": AI, "bdt": BDt, "bd": BD, "rq": RQ, "hq": HQ,
            "i96": I96}


_CACHE = {}

# constants dtype classes
_F16C = {"a1", "ai", "bd", "hq", "i96"}   # fp16 constants
_F32RC = {"bdt"}                          # f32r constants


def _build(work_bufs=3, rsb_n=3, d_rcopy=368, load_split=1,
           store_split=2, uu_pool=True):
    """d_rcopy: R-copy columns [0:d_rcopy] on ACT, rest on DVE.
    uu_pool: run the uu=tt*hq multiply on gpsimd instead of DVE."""
    import concourse.bacc as bacc
    import concourse.mybir as mybir
    import concourse.tile as tile

    F32 = mybir.dt.float32
    F32R = mybir.dt.float32r
    F16 = mybir.dt.float16
    AOT = mybir.AluOpType
    nc = bacc.Bacc("TRN2", target_bir_lowering=False, debug=False)

    x = nc.dram_tensor("x", [BPC, C, H, W], F32, kind="ExternalInput")
    xh = nc.dram_tensor("xh", [BPC, C, H, W], F16, kind="Internal")
    out = nc.dram_tensor("out", [BPC, C, H, W], F32, kind="ExternalOutput")

    def cdt(k):
        return F16 if k in _F16C else (F32R if k in _F32RC else F32)

    cd = {k: nc.dram_tensor(k, list(vv.shape), cdt(k), kind="ExternalInput")
          for k, vv in _np_consts().items()}

    x_flat = x.ap().rearrange("b c h w -> b (c h w)")
    xh_flat = xh.ap().rearrange("b c h w -> b (c h w)")
    xin_src = xh.ap().rearrange("b c (t g xx) w -> b c (g xx) t w",
                                t=NT, g=G, xx=XX)
    out_dst = out.ap().rearrange("b c (t g xx) w -> b c (g xx) t w",
                                 t=NT, g=G, xx=XX)

    with tile.TileContext(nc) as tc:
        # persistent SBUF state
        csb = {k: nc.alloc_sbuf_tensor(f"c_{k}", list(v.shape), cdt(k))
               for k, v in _np_consts().items()}
        xin = [nc.alloc_sbuf_tensor(f"xin{i}", [97, FREE], F16) for i in range(2)]
        rout = [nc.alloc_sbuf_tensor(f"rout{i}", [P96, FREE], F32) for i in range(2)]
        rsb = [nc.alloc_sbuf_tensor(f"rsb{i}", [P96, W], F16) for i in range(rsb_n)]
        zbias = nc.alloc_sbuf_tensor("zbias", [128, 1], F32)

        for k, t in csb.items():
            nc.sync.dma_start(out=t.ap(), in_=cd[k].ap())
        nc.vector.memset(zbias.ap(), 0.0)
        for i in range(2):
            nc.vector.memset(xin[i].ap()[96:97, :], 1.0)

        a1, ai = csb["a1"].ap(), csb["ai"].ap()
        bdt, bd = csb["bdt"].ap(), csb["bd"].ap()
        rq, hq = csb["rq"].ap(), csb["hq"].ap()
        i96 = csb["i96"].ap()
        zb = zbias.ap()

        with (
            tc.tile_pool(name="psT", bufs=3, space="PSUM") as psT,
            tc.tile_pool(name="psC", bufs=2, space="PSUM") as psC,
            tc.tile_pool(name="psR", bufs=2, space="PSUM") as psR,
            tc.tile_pool(name="psY", bufs=1, space="PSUM") as psY,
            tc.tile_pool(name="work", bufs=work_bufs) as work,
        ):
            stchunk = NT // store_split
            chn = C * H * W // load_split

            def cast_image(b):
                # one contiguous HBM->HBM fp32 -> fp16 cast (Pool SWDGE)
                for s0 in range(load_split):
                    nc.gpsimd.dma_start(
                        out=xh_flat[b, s0*chn:(s0+1)*chn],
                        in_=x_flat[b, s0*chn:(s0+1)*chn])

            def load_image(b):
                xv = xin[b % 2].ap()
                for c in range(CCH):
                    nc.sync.dma_start(
                        out=xv[c*32:(c+1)*32, :].rearrange(
                            "p (t w) -> p t w", t=NT),
                        in_=xin_src[b, c])

            def store_half(b, s0):
                ov = rout[b % 2].ap()
                for c in range(CCH):
                    nc.sync.dma_start(
                        out=out_dst[b, c, :, s0*stchunk:(s0+1)*stchunk],
                        in_=ov[c*32:(c+1)*32,
                               s0*stchunk*W:(s0+1)*stchunk*W].rearrange(
                            "p (t w) -> p t w", t=stchunk))

            # 5-stage software pipeline over the 64 (image, h-tile) items:
            #   it j:   MM1(j)x4 -> T_ps ; T-copy -> t_sb (ACT)
            #   it j+1: MM2(j) -> C_ps ; d15(j) DVE ; tanh(j) ACT -> fp16
            #   it j+2: uu(j) DVE 4x fp16
            #   it j+3: MM3(j)x4 -> R_ps ; R-copy -> rsb fp16 (Pool/DVE)
            #   it j+4: MM4(j)+ident -> Y_ps ; clip(j) DVE/Pool ; store
            items = [(b, t) for b in range(BPC) for t in range(NT)]
            NI = len(items)
            st = {}   # per-item live tiles

            for b in range(BPC):
                cast_image(b)
            load_image(0)
            if BPC > 1:
                load_image(1)

            for i in range(NI + 4):
                # PE ops first: all deps are >= 1 iteration old.
                if i < NI:
                    b, t = items[i]
                    xv = xin[b % 2].ap()
                    base = t * W
                    T_ps = psT.tile([128, 384], F32)
                    for j in range(4):
                        nc.tensor.matmul(
                            T_ps[:, 96*j:96*j+96],
                            xv[0:97, base+128*j:base+128*j+128],
                            a1, start=True, stop=True)
                    st[i] = {"T_ps": T_ps, "b": b, "t": t}
                if i - 1 >= 0 and i - 1 < NI:
                    e = st[i - 1]
                    C_ps = psC.tile([128, 384], F32)
                    nc.tensor.matmul(C_ps[:, :], bdt, e["t_sb"],
                                     start=True, stop=True)
                    e["C_ps"] = C_ps
                if i - 3 >= 0 and i - 3 < NI:
                    e = st[i - 3]
                    R_ps = psR.tile([P96, W], F32)
                    for j in range(4):
                        nc.tensor.matmul(
                            R_ps[:, 128*j:128*j+128],
                            e["uu"][:, 96*j:96*j+96],
                            bd, start=True, stop=True)
                    e["R_ps"] = R_ps
                if i - 4 >= 0 and i - 4 < NI:
                    e = st[i - 4]
                    xv4 = xin[e["b"] % 2].ap()
                    base4 = e["t"] * W
                    Y_ps = psY.tile([P96, W], F32)
                    nc.tensor.matmul(Y_ps[:, :], ai, e["rv"][0:P96, :],
                                     start=True, stop=False)
                    nc.tensor.matmul(Y_ps[:, :], i96,
                                     xv4[0:P96, base4:base4+W],
                                     start=False, stop=True)
                    e["Y_ps"] = Y_ps

                # ACT ops
                if i < NI:
                    e = st[i]
                    t_sb = work.tile([128, 384], F32R, tag="t_sb")
                    nc.scalar.copy(t_sb, e["T_ps"][:, :])
                    e["t_sb"] = t_sb
                if i - 1 >= 0 and i - 1 < NI:
                    e = st[i - 1]
                    # |d| = |C|/q <= 4.0/10 < 0.5 always => round(d) == 0,
                    # so tanh(15(d-round(d))) == tanh(RQ15*C) exactly.
                    d15 = work.tile([128, 384], F32, tag="d15")
                    nc.vector.tensor_tensor(d15, e["C_ps"][:, :], rq, AOT.mult)
                    tt = work.tile([128, 384], F16, tag="tt")
                    nc.scalar.activation(tt, d15,
                                         mybir.ActivationFunctionType.Tanh,
                                         bias=zb, scale=1.0)
                    e["tt"] = tt

                # quant scale uu = tt*hq, fp16 (gpsimd keeps DVE free; it is
                # the only SBUF-only op so the only Pool-eligible one)
                if i - 2 >= 0 and i - 2 < NI:
                    e = st[i - 2]
                    uu = work.tile([128, 384], F16, tag="uu")
                    (nc.gpsimd if uu_pool else nc.vector).tensor_tensor(
                        uu, e["tt"], hq, AOT.mult)
                    e["uu"] = uu

                # R-copy: PSUM -> SBUF fp16, column-split ACT/DVE
                if i - 3 >= 0 and i - 3 < NI:
                    e = st[i - 3]
                    rv = rsb[(i - 3) % rsb_n].ap()
                    if d_rcopy > 0:
                        nc.scalar.copy(rv[0:P96, 0:d_rcopy],
                                       e["R_ps"][:, 0:d_rcopy])
                    if d_rcopy < W:
                        nc.vector.tensor_scalar(
                            rv[0:P96, d_rcopy:W], e["R_ps"][:, d_rcopy:W],
                            0.0, None, AOT.add)
                    e["rv"] = rv

                # clip on DVE (PSUM-reading min+max: DVE only)
                if i - 4 >= 0 and i - 4 < NI:
                    e = st[i - 4]
                    ov = rout[e["b"] % 2].ap()
                    t4 = e["t"]
                    nc.vector.tensor_scalar(
                        ov[:, t4*W:(t4+1)*W], e["Y_ps"][:, :],
                        0.0, 1.0, AOT.max, AOT.min)
                    if t4 == NT - 1:
                        for s0 in range(store_split):
                            store_half(e["b"], s0)
                    del st[i - 4]

                # prefetch: the ident matmul reads xin 4 stages late, so
                # image b-1's last xin[(b+1)%2] read is emitted at iteration
                # 16b+3; trigger the overwrite strictly after that.
                if i < NI:
                    b, t = items[i]
                    if t == 4 and b >= 1 and b + 1 < BPC:
                        load_image(b + 1)
    nc.compile()
    return nc


def _get_nc(**kw):
    key = tuple(sorted(kw.items()))
    if key not in _CACHE:
        _CACHE[key] = _build(**kw)
    return _CACHE[key]


def kernel(x, trace=False, **kw):
    from concourse import bass_utils
    nc = _get_nc(**kw)
    consts = _np_consts()
    for k in _F16C:
        consts[k] = consts[k].astype(np.float16)
    x = np.ascontiguousarray(np.asarray(x), dtype=np.float32)
    in_maps = []
    for i in range(NCORES):
        m = {"x": x[i*BPC:(i+1)*BPC]}
        m.update(consts)
        in_maps.append(m)
    res = bass_utils.run_bass_kernel_spmd(
        nc, in_maps, core_ids=list(range(NCORES)), trace=trace)
    _CACHE["last"] = res
    return np.concatenate([r["out"] for r in res.results], axis=0)


def last_exec_time_ns():
    res = _CACHE.get("last")
    return None if res is None else res.exec_time_ns


# revision 37
# speedup vs baseline: 1.3768x; 1.2257x over previous
"""Differentiable-JPEG Trainium2 kernel (8-core data-parallel, full I/O).

Identity-split formulation: since IDCT2(DCT2(x)) == x and M_INV @ M_FWD == I
(to 6e-7), the reference reduces to

    out = clip(x + M_INV . IDCT2(corr), 0, 1),
    corr = 0.5 * q * tanh(15 * C / q),   C = DCT2(M_FWD.x + OFF - 0.5)

(|C/q| <= 4/10 < 0.5 so round(C/q) == 0 exactly). Only the small correction
term flows through the transform chain, so the chain runs in fp16/f32r; x
re-enters through an identity-weight matmul accumulated into MM4's PSUM,
which deletes the dequant add and all bias plumbing.

The input image is pre-converted fp32->fp16 per image by a single
contiguous HBM->HBM gpsimd DMA (only Pool software-DGE can cast in
flight), then loaded fp16; the fp16 x quantization (2.4e-4) is well
inside the 2e-2 gate and MM1 becomes a legal fp16 matmul.

Pipeline per 32-row x 512-col x 3-channel tile (96 packed partitions):
  MM1 (x4):  T = X^T @ A1        color + H-DCT (fp16)
  MM2:       C = BDt^T @ T       W-DCT (f32r, free 384)
  quant:     d15 = C*RQ (DVE) ; tt = tanh(d15) fp16 (ACT)
             uu = tt*HQ fp16 (DVE 4x mode)
  MM3 (x4):  R = uu^T @ BD       W-IDCT (fp16)
  MM4:       Y = AI^T @ R + I^T @ X   H-IDCT+color-inv plus identity
  clip:      out = min(max(Y,0),1)    column-split DVE/Pool

Batch dim (32) sharded 4-per-core across 8 NeuronCores; constants
replicated. PSUM->SBUF copies and clip are column-split across ACT/DVE/Pool
to balance engine busy time; stores ride the otherwise idle sync engine.
"""
import numpy as np

B, C, H, W = 32, 3, 512, 512
NCORES = 8
BPC = B // NCORES           # images per core
G, CCH, XX = 4, 3, 8        # 8-row groups per tile, channels, rows per block
P96 = G * CCH * XX          # 96 packed partitions
NT = H // 32                # 16 h-tiles per image
FREE = NT * W               # 8192 free elements per image buffer

QUALITY = 50.0
_LUM = np.array([[16,11,10,16,24,40,51,61],[12,12,14,19,26,58,60,55],[14,13,16,24,40,57,69,56],[14,17,22,29,51,87,80,62],[18,22,37,56,68,109,103,77],[24,35,55,64,81,104,113,92],[49,64,78,87,103,121,120,101],[72,92,95,98,112,100,103,99]], dtype=np.float32)
_CHR = np.array([[17,18,24,47,99,99,99,99],[18,21,26,66,99,99,99,99],[24,26,56,99,99,99,99,99],[47,66,99,99,99,99,99,99],[99,99,99,99,99,99,99,99],[99,99,99,99,99,99,99,99],[99,99,99,99,99,99,99,99],[99,99,99,99,99,99,99,99]], dtype=np.float32)


def _scaled_qtable(base, qf):
    qf = max(1.0, min(100.0, qf))
    s = 5000.0 / qf if qf < 50 else 200.0 - 2.0 * qf
    return np.maximum(np.floor((base * s + 50.0) / 100.0), 1.0)


def _np_consts():
    qtab = np.stack([_scaled_qtable(_LUM, QUALITY), _scaled_qtable(_CHR, QUALITY),
                     _scaled_qtable(_CHR, QUALITY)]).astype(np.float32)  # [c,u,v]
    u8 = np.arange(8)[:, None]
    x8 = np.arange(8)[None, :]
    cu = np.where(u8 == 0, 1.0 / np.sqrt(2.0), 1.0)
    D = (0.5 * cu * np.cos((2 * x8 + 1) * u8 * np.pi / 16.0)).astype(np.float32)
    MFWD = np.array([[0.299, 0.587, 0.114], [-0.168736, -0.331264, 0.5],
                     [0.5, -0.418688, -0.081312]], np.float32)
    MINV = np.array([[1.0, 0.0, 1.402], [1.0, -0.344136, -0.714136],
                     [1.0, 1.772, 0.0]], np.float32)

    A1 = np.zeros((97, 96), np.float32)
    AI = np.zeros((96, 96), np.float32)
    for g in range(G):
        for c in range(CCH):
            for c2 in range(CCH):
                p0 = c*32 + g*8
                n0 = c2*32 + g*8
                # A1[p=(c,g,xx), n=(c2,g,u)] = MFWD[c2,c] * D[u,xx]
                A1[p0:p0+8, n0:n0+8] = MFWD[c2, c] * D.T
                # AI[k=(c2,g,u), m=(c,g,xx)] = MINV[c,c2] * D[u,xx]
                AI[n0:n0+8, p0:p0+8] = MINV[c, c2] * D
        A1[96, g*8] = -np.sqrt(2.0)     # (c2=Y, u=0): forward -0.5 pixel bias

    BDt = np.zeros((128, 128), np.float32)
    for a in range(16):
        BDt[8*a:8*a+8, 8*a:8*a+8] = D.T
    BD = np.ascontiguousarray(BDt.T)

    RQ = np.zeros((128, 384), np.float32)
    HQ = np.zeros((128, 384), np.float32)
    v = np.arange(128) % 8
    for j in range(4):
        for c in range(CCH):
            for g in range(G):
                for u in range(XX):
                    col = j*96 + c*32 + g*8 + u
                    RQ[:, col] = 15.0 / qtab[c, u, v]
                    HQ[:, col] = 0.5 * qtab[c, u, v]
    I96 = np.eye(96, dtype=np.float32)
    ONES = np.ones((1, FREE), np.float32)
    HQ2 = np.concatenate([HQ, HQ], axis=1)
    return {"a1": A1, "ai": AI, "bdt": BDt, "bd": BD, "rq": RQ, "hq": HQ2,
            "i96": I96, "ones": ONES}


_CACHE = {}

# constants dtype classes
_F16C = {"a1", "bd", "hq", "i96", "ones"}   # fp16 constants
_F32RC = {"bdt", "ai"}                      # f32r constants


def _build(work_bufs=3, rsb_n=3, d_rcopy=424, d_rdma=False, load_split=2,
           store_split=4, uu_pool=True, ablate_dma=False, xin_n=4,
           cast_t=4, load_t=8, rout_n=3, cast0_split=6, sched="v1"):
    """d_rcopy: R-copy columns [0:d_rcopy] on ACT; rest on DVE, or on a
    sync-engine DMA when d_rdma (PSUM->SBUF, rides spare DMA bandwidth).
    uu_pool: run the uu=tt*hq multiply on gpsimd instead of DVE."""
    import concourse.bacc as bacc
    import concourse.mybir as mybir
    import concourse.tile as tile

    F32 = mybir.dt.float32
    F32R = mybir.dt.float32r
    F16 = mybir.dt.float16
    AOT = mybir.AluOpType
    nc = bacc.Bacc("TRN2", target_bir_lowering=False, debug=False)

    x = nc.dram_tensor("x", [BPC, C, H, W], F32, kind="ExternalInput")
    XHN = 3
    xh = nc.dram_tensor("xh", [XHN, C, H, W], F16, kind="Internal")
    out = nc.dram_tensor("out", [BPC, C, H, W], F32, kind="ExternalOutput")

    def cdt(k):
        return F16 if k in _F16C else (F32R if k in _F32RC else F32)

    cd = {k: nc.dram_tensor(k, list(vv.shape), cdt(k), kind="ExternalInput")
          for k, vv in _np_consts().items()}

    x_flat = x.ap().rearrange("b c h w -> b (c h w)")
    xh_flat = xh.ap().rearrange("b c h w -> b (c h w)")
    xin_src = xh.ap().rearrange("b c (t g xx) w -> b c (g xx) t w",
                                t=NT, g=G, xx=XX)
    out_dst = out.ap().rearrange("b c (t g xx) w -> b c (g xx) t w",
                                 t=NT, g=G, xx=XX)

    with tile.TileContext(nc) as tc:
        # persistent SBUF state
        csb = {k: nc.alloc_sbuf_tensor(f"c_{k}", list(v.shape), cdt(k))
               for k, v in _np_consts().items() if k != "ones"}
        xin = [nc.alloc_sbuf_tensor(f"xin{i}", [97, FREE], F16)
               for i in range(xin_n)]
        rout = [nc.alloc_sbuf_tensor(f"rout{i}", [P96, FREE], F32)
               for i in range(rout_n)]
        rsb = [nc.alloc_sbuf_tensor(f"rsb{i}", [P96, W], F32R)
               for i in range(rsb_n)]
        zbias = nc.alloc_sbuf_tensor("zbias", [128, 1], F32)


        a1, ai = csb["a1"].ap(), csb["ai"].ap()
        bdt, bd = csb["bdt"].ap(), csb["bd"].ap()
        rq, hq = csb["rq"].ap(), csb["hq"].ap()
        i96 = csb["i96"].ap()
        zb = zbias.ap()

        with (
            tc.tile_pool(name="psT", bufs=2, space="PSUM") as psT,
            tc.tile_pool(name="psC", bufs=2, space="PSUM") as psC,
            tc.tile_pool(name="psR", bufs=2, space="PSUM") as psR,
            tc.tile_pool(name="psY", bufs=2, space="PSUM") as psY,
            tc.tile_pool(name="work", bufs=work_bufs) as work,
        ):
            stchunk = NT // store_split
            chn = C * H * W // load_split

            def cast_image(b, split=1):
                # HBM->HBM fp32 -> fp16 cast (Pool SWDGE). split=6 chunks
                # per (half, channel), half-major, so the first h-tiles'
                # rows across all channels arrive first and MM1 can start
                # while the second half is still in flight.
                if ablate_dma:
                    return
                if split == 1:
                    nc.gpsimd.dma_start(out=xh_flat[b % XHN, :],
                                        in_=x_flat[b, :])
                    return
                hw2 = H * W // 2
                for s0 in range(2):
                    for c in range(CCH):
                        lo = c * H * W + s0 * hw2
                        nc.gpsimd.dma_start(
                            out=xh_flat[b % XHN, lo:lo+hw2],
                            in_=x_flat[b, lo:lo+hw2])

            def load_image(b):
                if ablate_dma:
                    return
                xv = xin[b % xin_n].ap()
                ltc = NT // load_split
                for s0 in range(load_split):
                    for c in range(CCH):
                        nc.sync.dma_start(
                            out=xv[c*32:(c+1)*32,
                                   s0*ltc*W:(s0+1)*ltc*W].rearrange(
                                "p (t w) -> p t w", t=ltc),
                            in_=xin_src[b % XHN, c, :, s0*ltc:(s0+1)*ltc])

            def store_half(b, s0):
                if ablate_dma:
                    return
                ov = rout[b % rout_n].ap()
                for c in range(CCH):
                    nc.sync.dma_start(
                        out=out_dst[b, c, :, s0*stchunk:(s0+1)*stchunk],
                        in_=ov[c*32:(c+1)*32,
                               s0*stchunk*W:(s0+1)*stchunk*W].rearrange(
                            "p (t w) -> p t w", t=stchunk))

            # 10-stage software pipeline over the 64 (image, h-tile) items:
            # exactly one dependency hop per emission iteration, so no
            # iteration carries an intra-iteration serial chain:
            #   i:   MM1(k)x4 -> T_ps        (PE)
            #   i+1: T-copy(k) -> t_sb       (ACT)
            #   i+2: MM2(k) -> C_ps          (PE)
            #   i+3: d15(k) = C*RQ           (DVE)
            #   i+4: tanh(k) -> tt fp16      (ACT)
            #   i+5: uu(k) = tt*HQ fp16      (Pool)
            #   i+6: MM3(k)x4 -> R_ps        (PE)
            #   i+7: R-copy(k) -> rsb        (ACT + sync-DMA split)
            #   i+8: MM4(k)+ident -> Y_ps    (PE)
            #   i+9: clip(k) -> rout; store  (DVE)
            items = [(b, t) for b in range(BPC) for t in range(NT)]
            NI = len(items)
            st = {}   # per-item live tiles

            # image 0's cast split per (channel, half) so load(0) chases
            # it chunk by chunk; later casts whole. All cast preps run
            # during the fill while Pool is otherwise idle, in emission
            # order, which also orders their transfers behind load(0)'s.
            cast_image(0, split=cast0_split)
            load_image(0)
            nc.sync.dma_start(out=xin[0].ap()[96:97, :],
                              in_=cd["ones"].ap())
            for k, cst in csb.items():
                nc.sync.dma_start(out=cst.ap(), in_=cd[k].ap())
            # ones rows DMA'd, not memset (a [1,8192] memset costs
            # free-size cycles on DVE)
            for ii in range(1, xin_n):
                nc.sync.dma_start(out=xin[ii].ap()[96:97, :],
                                  in_=cd["ones"].ap())
            nc.vector.memset(zbias.ap(), 0.0)
            if sched == "v1":
                for b2 in range(1, BPC):
                    cast_image(b2)
                if BPC > 1:
                    load_image(1)


            def live(j):
                return j >= 0 and j < NI

            for i in range(NI + 11):
                # prefetch first so loads queue ahead of this iteration's
                # stores on the SP/DMA path; xin is 3-deep so image b+2's
                # load can start while image b's idents still read xin[b%3]
                if i < NI:
                    b, t = items[i]
                    if sched == "v1":
                        if t == load_t and b + 2 < BPC:
                            load_image(b + 2)
                    else:
                        if t == cast_t and b + 1 < BPC:
                            cast_image(b + 1)
                        if t == load_t and b + 1 < BPC:
                            load_image(b + 1)

                # PE ops: all deps are >= 1 iteration old.
                if i < NI:
                    b, t = items[i]
                    xv = xin[b % xin_n].ap()
                    base = t * W
                    T_ps = psT.tile([128, 384], F32)
                    for j in range(4):
                        nc.tensor.matmul(
                            T_ps[:, 96*j:96*j+96],
                            xv[0:97, base+128*j:base+128*j+128],
                            a1, start=True, stop=True)
                    st[i] = {"T_ps": T_ps, "b": b, "t": t}
                if live(i - 2):
                    e = st[i - 2]
                    C_ps = psC.tile([128, 384], F32)
                    nc.tensor.matmul(C_ps[:, :], bdt, e["t_sb"],
                                     start=True, stop=True)
                    e["C_ps"] = C_ps
                if live(i - 7) and (i - 7) % 2 == 1:
                    for k in (i - 8, i - 7):
                        e = st[k]
                        R_ps = psR.tile([P96, W], F32)
                        for j in range(4):
                            nc.tensor.matmul(
                                R_ps[:, 128*j:128*j+128],
                                e["uu"][:, 96*j:96*j+96],
                                bd, start=True, stop=True)
                        e["R_ps"] = R_ps
                if live(i - 9) and (i - 9) % 2 == 1:
                    for k in (i - 10, i - 9):
                        e = st[k]
                        xv8 = xin[e["b"] % xin_n].ap()
                        base8 = e["t"] * W
                        Y_ps = psY.tile([P96, W], F32)
                        nc.tensor.matmul(Y_ps[:, :], ai, e["rv"][0:P96, :],
                                         start=True, stop=False)
                        nc.tensor.matmul(Y_ps[:, :], i96,
                                         xv8[0:P96, base8:base8+W],
                                         start=False, stop=True)
                        e["Y_ps"] = Y_ps

                # ACT: T-copy one hop after MM1
                if live(i - 1):
                    e = st[i - 1]
                    t_sb = work.tile([128, 384], F32R, tag="t_sb")
                    nc.scalar.copy(t_sb, e["T_ps"][:, :])
                    e["t_sb"] = t_sb
                # DVE: d15 one hop after MM2; item k writes half k%2 of a
                # pair-shared [128,768] tile so tanh/uu run once per pair
                # (halves the per-op SBUF-access setup cost)
                if live(i - 3):
                    e = st[i - 3]
                    k3 = i - 3
                    # |d| = |C|/q <= 4.0/10 < 0.5 always => round(d) == 0,
                    # so tanh(15(d-round(d))) == tanh(RQ15*C) exactly.
                    if k3 % 2 == 0:
                        d2 = work.tile([128, 768], F32, tag="d2")
                        e["d2"] = d2
                    else:
                        d2 = st[k3 - 1]["d2"]
                        e["d2"] = d2
                    nc.vector.tensor_tensor(d2[:, 384*(k3 % 2):384*(k3 % 2)+384],
                                            e["C_ps"][:, :], rq, AOT.mult)
                # ACT: tanh once per pair, after the odd d15
                if live(i - 4):
                    k4 = i - 4
                    e = st[k4]
                    if k4 % 2 == 1:
                        t2 = work.tile([128, 768], F16, tag="t2")
                        nc.scalar.activation(t2, e["d2"],
                                             mybir.ActivationFunctionType.Tanh,
                                             bias=zb, scale=1.0)
                        e["t2"] = t2
                        if k4 % 2 == 1:
                            st[k4 - 1]["t2"] = t2
                # Pool: uu = tt*hq fp16, once per pair
                if live(i - 5):
                    k5 = i - 5
                    e = st[k5]
                    if k5 % 2 == 1:
                        u2 = work.tile([128, 768], F16, tag="u2")
                        (nc.gpsimd if uu_pool else nc.vector).tensor_tensor(
                            u2, e["t2"], hq, AOT.mult)
                        e["uu"] = u2[:, 384:768]
                        st[k5 - 1]["uu"] = u2[:, 0:384]
                # R-copy: PSUM -> SBUF, ACT columns [0:d_rcopy], rest DVE
                if live(i - 8) and (i - 8) % 2 == 1:
                    for k in (i - 9, i - 8):
                        e = st[k]
                        rv = rsb[k % rsb_n].ap()
                        if d_rcopy > 0:
                            nc.scalar.copy(rv[0:P96, 0:d_rcopy],
                                           e["R_ps"][:, 0:d_rcopy])
                        if d_rcopy < W:
                            nc.vector.tensor_scalar(
                                rv[0:P96, d_rcopy:W],
                                e["R_ps"][:, d_rcopy:W],
                                0.0, None, AOT.add)
                        e["rv"] = rv
                # DVE: clip + store
                if live(i - 10) and (i - 10) % 2 == 1:
                    for k in (i - 11, i - 10):
                        e = st[k]
                        ov = rout[e["b"] % rout_n].ap()
                        t9 = e["t"]
                        nc.vector.tensor_scalar(
                            ov[:, t9*W:(t9+1)*W], e["Y_ps"][:, :],
                            0.0, 1.0, AOT.max, AOT.min)
                        if (t9 + 1) % stchunk == 0:
                            store_half(e["b"], (t9 + 1) // stchunk - 1)
                        del st[k]

    nc.compile()
    return nc


def _get_nc(**kw):
    key = tuple(sorted(kw.items()))
    if key not in _CACHE:
        _CACHE[key] = _build(**kw)
    return _CACHE[key]


def kernel(x, trace=False, **kw):
    from concourse import bass_utils
    nc = _get_nc(**kw)
    consts = _np_consts()
    for k in _F16C:
        consts[k] = consts[k].astype(np.float16)
    x = np.ascontiguousarray(np.asarray(x), dtype=np.float32)
    in_maps = []
    for i in range(NCORES):
        m = {"x": x[i*BPC:(i+1)*BPC]}
        m.update(consts)
        in_maps.append(m)
    res = bass_utils.run_bass_kernel_spmd(
        nc, in_maps, core_ids=list(range(NCORES)), trace=trace)
    _CACHE["last"] = res
    return np.concatenate([r["out"] for r in res.results], axis=0)


def last_exec_time_ns():
    res = _CACHE.get("last")
    return None if res is None else res.exec_time_ns
